# revision 25
# baseline (speedup 1.0000x reference)
"""DAGNConv (GNN message passing) Trainium2 kernel.

Strategy (8 NeuronCores, SPMD):
  - Host bin-packs nodes into 784 tiles of <=128 nodes (LPT on degree) so
    every tile holds ~640 edges -> l_cap=640 (5 chunks of 128 edge slots).
    Each core owns 98 tiles; segment softmax is core-local.
  - Scores (iter 1): per-node tables s_h/s_t (device matmul over entity
    tiles) and s_r (relation tiles).  Per-edge assembly:
      s_t rides the tail gather (combined [ent|s_t] bf16 row),
      s_h via host-shipped transposed head one-hot (bf16 matmul),
      s_r via host-shipped relation one-hot (bf16 matmul).
    Softmax denominator folded into the message matmul (4 extra columns);
    attention normalization folded into a per-node scale.
  - Power iterations: per 128-edge chunk, indirect-DMA row gather of
    Z[tails] (one row per partition), then segment-sum via one-hot matmul
    built on device (is_equal vs iota).  All Z state bf16.
  - Between iterations: the [12544, 256] Z shard AllGather is split into
    7 group collectives (14 tiles each) so they overlap tile compute.
  - Output: Z5 @ W_o folded into iteration 5 (PE transpose + matmul).
"""

import os
import sys

import numpy as np

for _p in ("/opt/trn_rl_repo",):
    if _p not in sys.path and os.path.isdir(_p):
        sys.path.insert(0, _p)

P = 128
N_ENT = 100000
N_EDGE = 500000
N_REL = 200
DIM = 64
HEADS = 4
HD = HEADS * DIM  # 256
CROW = DIM + HEADS  # combined [ent | s_t] row length (68)
POW_ITER = 5
ALPHA = 0.1
LEAKY = 0.01
EPS = 1e-16
NCORES = 8
NTILES = 98          # tiles per core
NBINS = NCORES * NTILES
NPS = NTILES * P     # padded nodes per core (12544)
NPT = NCORES * NPS   # padded total nodes (100352)
# collective groups per iteration (tile boundaries, uneven: last group
# small so the iteration-boundary stall on its allgather is short)
GBOUND = [0, 16, 32, 48, 64, 80, 94, 98]
RB = [b * P for b in GBOUND]  # row boundaries per core


class Cfg:
    def __init__(self, l_cap):
        assert l_cap % P == 0
        self.l_cap = l_cap
        self.ch = l_cap // P          # chunks per tile
        self.nchunk = NTILES * self.ch  # chunks per core


def _balance_nodes(heads):
    """LPT bin-packing of nodes into NBINS tiles (<=128 nodes each),
    minimizing max edges per tile.  Returns newpos[orig_node] (global
    padded row) and per-bin node lists."""
    import heapq

    deg = np.bincount(heads, minlength=N_ENT)
    order = np.argsort(-deg, kind="stable")
    heap = [(0, b) for b in range(NBINS)]
    heapq.heapify(heap)
    bin_nodes = [[] for _ in range(NBINS)]
    bin_load = np.zeros(NBINS, np.int64)
    for n in order:
        d = int(deg[n])
        while True:
            load, b = heapq.heappop(heap)
            if len(bin_nodes[b]) < P:
                bin_nodes[b].append(n)
                bin_load[b] = load + d
                heapq.heappush(heap, (load + d, b))
                break
            # full bins simply stay out of the heap
    newpos = np.empty(N_ENT, np.int64)
    for b in range(NBINS):
        for i, n in enumerate(bin_nodes[b]):
            newpos[n] = b * P + i
    return newpos, int(bin_load.max())


def host_prep(cfg, entity_embed, relation_embed, edge_index, edge_type):
    import ml_dtypes

    bf16 = ml_dtypes.bfloat16
    h = np.asarray(edge_index[0], dtype=np.int64)
    t = np.asarray(edge_index[1], dtype=np.int64)
    r = np.asarray(edge_type, dtype=np.int64)

    newpos, max_load = _balance_nodes(h)
    l_cap = -(-max_load // P) * P
    if l_cap != cfg.l_cap:
        cfg = Cfg(l_cap)
    CH = cfg.ch

    # per-edge new head row, sorted so each tile's edges are contiguous
    hn = newpos[h]
    perm = np.argsort(hn, kind="stable")
    hs, ts, rs = hn[perm], t[perm], r[perm]
    tn = newpos[ts]  # new padded tail row (plain layout)

    # group-major z_full row for each tail (uneven group bounds)
    cc = tn // NPS
    ll = tn % NPS
    rb = np.asarray(RB, np.int64)
    g = np.searchsorted(rb, ll, side="right") - 1
    tz = NCORES * rb[g] + cc * (rb[g + 1] - rb[g]) + (ll - rb[g])

    ent = np.asarray(entity_embed, dtype=np.float32)
    ent_new = np.zeros((NPT, DIM), np.float32)
    ent_new[newpos] = ent
    relpad = np.zeros((2 * P, DIM), np.float32)
    relpad[:N_REL] = np.asarray(relation_embed, np.float32)
    relpad_b = relpad.astype(bf16)

    cores = []
    tile_of_edge = hs // P          # global tile id per edge
    # slot within tile
    tile_start = np.searchsorted(tile_of_edge, np.arange(NBINS))
    for c in range(NCORES):
        hrel = np.full((P, cfg.nchunk), -1.0, np.float32)
        tti = np.zeros((P, cfg.nchunk), np.int32)
        s1h = np.zeros((P, cfg.nchunk * P), bf16)
        h1h = np.zeros((P, cfg.nchunk * P), bf16)
        r1h = np.zeros((2 * P, cfg.nchunk * P), bf16)
        for i in range(NTILES):
            b = c * NTILES + i
            lo = int(tile_start[b])
            hi = int(tile_start[b + 1]) if b + 1 < NBINS else len(hs)
            cnt = hi - lo
            if cnt == 0:
                continue
            j = np.arange(cnt)
            cols = i * CH + (j // P)
            parts = j % P
            hloc = (hs[lo:hi] - b * P).astype(np.int64)
            hrel[parts, cols] = hloc.astype(np.float32)
            tti[parts, cols] = tz[lo:hi].astype(np.int32)
            s1h[parts, cols * P + hloc] = 1.0
            h1h[hloc, cols * P + parts] = 1.0
            r1h[rs[lo:hi], cols * P + parts] = 1.0
        entloc = ent_new[c * NPS:(c + 1) * NPS].astype(bf16)
        cores.append(dict(hrel=hrel, tti=tti, s1h=s1h,
                          h1h=h1h, r1h=r1h, entloc=entloc))
    return cfg, cores, relpad_b, newpos


def build_program(cfg):
    import concourse.bass as bass
    import concourse.bacc as bacc
    import concourse.mybir as mybir
    from concourse.masks import make_identity
    from concourse.tile import TileContext

    f32 = mybir.dt.float32
    i32 = mybir.dt.int32
    bf16 = mybir.dt.bfloat16
    AF = mybir.ActivationFunctionType
    OP = mybir.AluOpType
    AX = mybir.AxisListType
    H, D = HEADS, DIM
    CH = cfg.ch
    NCK = cfg.nchunk
    MW = HD + H  # message matmul width in iter 1 (260)

    nc = bacc.Bacc("TRN2", target_bir_lowering=False, debug=False,
                   num_devices=NCORES)

    # ---- I/O ----
    entloc_d = nc.dram_tensor("entloc", [NPS, D], bf16, kind="ExternalInput")
    relpad_d = nc.dram_tensor("relpad", [2 * P, D], bf16, kind="ExternalInput")
    wht_d = nc.dram_tensor("wht", [D, 2 * HD], bf16, kind="ExternalInput")
    wr_d = nc.dram_tensor("wr", [D, HD], bf16, kind="ExternalInput")
    wo_d = nc.dram_tensor("wo", [P, 2 * D], bf16, kind="ExternalInput")
    attht_d = nc.dram_tensor("attht", [P, 2 * HD], bf16, kind="ExternalInput")
    attr_d = nc.dram_tensor("attr", [P, HD], bf16, kind="ExternalInput")
    tti_d = nc.dram_tensor("tti", [P, NCK], i32, kind="ExternalInput")
    s1h_d = nc.dram_tensor("s1h", [P, NCK * P], bf16, kind="ExternalInput")
    h1h_d = nc.dram_tensor("h1h", [P, NCK * P], bf16, kind="ExternalInput")
    r1h_d = nc.dram_tensor("r1h", [2 * P, NCK * P], bf16, kind="ExternalInput")
    out_d = nc.dram_tensor("out", [NPS, D], f32, kind="ExternalOutput")

    # ---- internal DRAM ----
    # comb rows are group-major (same layout as z_full) so the per-group
    # [ent|s_t] AllGather writes land contiguously.
    comb = nc.dram_tensor("comb", [NPT, CROW], bf16, addr_space="Shared")
    st68 = nc.dram_tensor("st68", [NPS, CROW], bf16)
    z_shard = nc.dram_tensor("z_shard", [NPS, HD], bf16)
    z_full = [nc.dram_tensor(f"z_full{i}", [NPT, HD], bf16,
                             addr_space="Shared") for i in range(2)]
    rg = [list(range(NCORES))]

    with TileContext(nc) as tc:
        with (
            tc.tile_pool(name="const", bufs=1) as cp,
            tc.tile_pool(name="work", bufs=3) as wk,
            tc.tile_pool(name="small", bufs=4) as sm,
            tc.tile_pool(name="zg", bufs=4) as zgp,
            tc.tile_pool(name="msg", bufs=3) as msgp,
            tc.tile_pool(name="sone", bufs=3) as sop,
            tc.tile_pool(name="oneh", bufs=4) as ohp,
            tc.tile_pool(name="ppA", bufs=2, space="PSUM") as ppA,
            tc.tile_pool(name="ppB", bufs=2, space="PSUM") as ppB,
            tc.tile_pool(name="ppC", bufs=2, space="PSUM") as ppC,
            tc.tile_pool(name="ppD", bufs=2, space="PSUM") as ppD,
        ):
            # ---- constants ----
            ident = cp.tile([P, P], f32, tag="ident")
            make_identity(nc, ident[:])
            identb = cp.tile([P, P], bf16, tag="identb")
            make_identity(nc, identb[:])
            def load_const(dram, shape, tag, dt=bf16):
                t = cp.tile(shape, dt, tag=tag)
                nc.sync.dma_start(t[:], dram[:, :])
                return t

            wht_t = load_const(wht_d, [D, 2 * HD], "wht")
            wr_t = load_const(wr_d, [D, HD], "wr")
            wo_t = load_const(wo_d, [P, 2 * D], "wo")
            attht_t = load_const(attht_d, [P, 2 * HD], "attht")
            attr_t = load_const(attr_d, [P, HD], "attr")
            tti_t = load_const(tti_d, [P, NCK], "tti", i32)

            w_sb = cp.tile([P, NCK * H], bf16, tag="w")
            inv_sb = cp.tile([P, NTILES * H], f32, tag="inv")
            sh_all = cp.tile([P, NTILES * H], bf16, tag="sh_all")
            sr_b = cp.tile([P, 2 * H], bf16, tag="sr_b")
            aent = cp.tile([P, NTILES * D], bf16, tag="aent")

            # ---- phase 1: node/relation score tables ----
            def table_pass(src_d, n_tiles, W_t, att_t, width, sink):
                for i in range(n_tiles):
                    ent = wk.tile([P, D], bf16, tag="ent")
                    nc.sync.dma_start(ent[:], src_d[i * P:(i + 1) * P, :])
                    tp = ppB.tile([P, P], bf16, tag="tp")
                    nc.tensor.transpose(out=tp[:D, :], in_=ent[:, :],
                                        identity=identb[:])
                    entT = wk.tile([P, P], bf16, tag="entT")
                    nc.scalar.activation(entT[:D, :], tp[:D, :], AF.Copy)
                    pj = ppA.tile([P, 2 * HD], f32, tag="mm")
                    nc.tensor.matmul(pj[:, :width], lhsT=entT[:D, :],
                                     rhs=W_t[:, :], start=True, stop=True)
                    th_ = wk.tile([P, 2 * HD], bf16, tag="tanh")
                    nc.scalar.activation(th_[:, :width], pj[:, :width],
                                         AF.Tanh)
                    pr = wk.tile([P, 2 * HD], f32, tag="prod")
                    nc.vector.tensor_tensor(out=pr[:, :width],
                                            in0=th_[:, :width],
                                            in1=att_t[:], op=OP.mult)
                    s_ = sm.tile([P, 2 * H], f32, tag="s8")
                    nc.vector.tensor_reduce(
                        out=s_[:, :width // D],
                        in_=pr[:, :width].rearrange("p (h d) -> p h d", d=D),
                        axis=AX.X, op=OP.add)
                    sink(i, ent, s_)

            def ent_sink(i, ent, s8):
                nc.scalar.activation(aent[:, i * D:(i + 1) * D], ent[:],
                                     AF.Copy, scale=ALPHA)
                nc.vector.tensor_copy(sh_all[:, i * H:(i + 1) * H],
                                      s8[:, 0:H])
                # assemble [ent | s_t] row block, ship to st68 shard
                cb = wk.tile([P, CROW], bf16, tag="cb")
                nc.vector.tensor_copy(cb[:, 0:D], ent[:])
                nc.vector.tensor_copy(cb[:, D:CROW], s8[:, H:2 * H])
                nc.scalar.dma_start(st68[i * P:(i + 1) * P, :], cb[:])
                if i + 1 in GBOUND:
                    g = GBOUND.index(i + 1) - 1
                    nc.gpsimd.collective_compute(
                        "AllGather", mybir.AluOpType.bypass,
                        replica_groups=rg,
                        ins=[st68[RB[g]:RB[g + 1], :].opt()],
                        outs=[comb[NCORES * RB[g]:NCORES * RB[g + 1],
                                   :].opt()])

            def rel_sink(i, ent, s4):
                nc.vector.tensor_copy(sr_b[:, i * H:(i + 1) * H], s4[:, 0:H])

            table_pass(entloc_d, NTILES, wht_t, attht_t, 2 * HD, ent_sink)
            table_pass(relpad_d, 2, wr_t, attr_t, HD, rel_sink)

            import concourse.bass as _b

            # ---- power iterations ----
            for k in range(1, POW_ITER + 1):
                first = k == 1
                last = k == POW_ITER
                src = comb if first else z_full[(k - 2) % 2]
                rowlen = CROW if first else HD
                for i in range(NTILES):
                    zg = zgp.tile([P, CH * rowlen], bf16, tag="zg")
                    for j in range(CH):
                        nc.gpsimd.indirect_dma_start(
                            out=zg[:, j * rowlen:(j + 1) * rowlen],
                            out_offset=None, in_=src[:, :],
                            in_offset=_b.IndirectOffsetOnAxis(
                                ap=tti_t[:, i * CH + j:i * CH + j + 1],
                                axis=0))
                    # one-hot S^T for all chunks of this tile: [P, CH*P]
                    s6 = sop.tile([P, CH * P], bf16, tag="s")
                    nc.sync.dma_start(
                        s6[:], s1h_d[:, i * CH * P:(i + 1) * CH * P])
                    mw = MW if first else HD
                    if first:
                        # per-edge scores: s_h + s_r via one-hot matmuls
                        # (one-hot blocks loaded per TILE, not per chunk)
                        oh = ohp.tile([P, CH * P], bf16, tag="oh")
                        nc.sync.dma_start(
                            oh[:], h1h_d[:, i * CH * P:(i + 1) * CH * P])
                        r1a = ohp.tile([P, CH * P], bf16, tag="r1a")
                        nc.sync.dma_start(
                            r1a[:], r1h_d[0:P, i * CH * P:(i + 1) * CH * P])
                        r1b = ohp.tile([P, CH * P], bf16, tag="r1b")
                        nc.sync.dma_start(
                            r1b[:], r1h_d[P:2 * P,
                                          i * CH * P:(i + 1) * CH * P])
                        shr = ppC.tile([P, D], f32, tag="shr")
                        for j in range(CH):
                            nc.tensor.matmul(
                                shr[:, j * H:(j + 1) * H],
                                lhsT=oh[:, j * P:(j + 1) * P],
                                rhs=sh_all[:, i * H:(i + 1) * H],
                                start=True, stop=False)
                            nc.tensor.matmul(
                                shr[:, j * H:(j + 1) * H],
                                lhsT=r1a[:, j * P:(j + 1) * P],
                                rhs=sr_b[:, 0:H],
                                start=False, stop=False)
                            nc.tensor.matmul(
                                shr[:, j * H:(j + 1) * H],
                                lhsT=r1b[:, j * P:(j + 1) * P],
                                rhs=sr_b[:, H:2 * H],
                                start=False, stop=True)
                        # scores for whole tile: add s_t, leaky, exp
                        stf = sm.tile([P, CH * H], f32, tag="stf")
                        nc.vector.tensor_copy(
                            stf[:].rearrange("p (c h) -> p c h", c=CH),
                            zg[:].rearrange("p (c r) -> p c r", c=CH)
                            [:, :, D:CROW])
                        sc = sm.tile([P, CH * H], f32, tag="sc")
                        nc.vector.tensor_tensor(out=sc[:],
                                                in0=shr[:, 0:CH * H],
                                                in1=stf[:], op=OP.add)
                        sc2 = sm.tile([P, CH * H], f32, tag="sc2")
                        nc.vector.tensor_scalar_mul(sc2[:], sc[:], LEAKY)
                        nc.vector.tensor_tensor(out=sc[:], in0=sc[:],
                                                in1=sc2[:], op=OP.max)
                        nc.scalar.activation(
                            w_sb[:, i * CH * H:(i + 1) * CH * H], sc[:],
                            AF.Exp)
                    # messages for the whole tile
                    msg = msgp.tile([P, CH * MW], bf16, tag="msg")
                    wap = (w_sb[:, i * CH * H:(i + 1) * CH * H]
                           .rearrange("p (c h o) -> p c h o", c=CH, h=H)
                           .to_broadcast([P, CH, H, D]))
                    mview = (msg[:, 0:CH * mw]
                             .rearrange("p (c x) -> p c x", c=CH)
                             [:, :, 0:HD]
                             .rearrange("p c (h d) -> p c h d", h=H))
                    if first:
                        zs = (zg[:].rearrange("p (c r) -> p c r", c=CH)
                              [:, :, 0:D]
                              .rearrange("p c (o d) -> p c o d", o=1)
                              .to_broadcast([P, CH, H, D]))
                    else:
                        zs = zg[:].rearrange("p (c h d) -> p c h d",
                                             c=CH, h=H)
                    nc.vector.tensor_tensor(out=mview, in0=zs, in1=wap,
                                            op=OP.mult)
                    if first:
                        # denominator columns: msg[:, c*MW+HD : c*MW+MW] = w
                        nc.vector.tensor_copy(
                            (msg[:, 0:CH * MW]
                             .rearrange("p (c x) -> p c x", c=CH)
                             [:, :, HD:MW]),
                            (w_sb[:, i * CH * H:(i + 1) * CH * H]
                             .rearrange("p (c h) -> p c h", c=CH)))
                    ps = ppD.tile([P, MW], f32, tag="mm")
                    for j in range(CH):
                        nc.tensor.matmul(
                            ps[:, :mw], lhsT=s6[:, j * P:(j + 1) * P],
                            rhs=msg[:, j * mw:(j + 1) * mw],
                            start=(j == 0), stop=(j == CH - 1))
                    if first:
                        d1 = sm.tile([P, H], f32, tag="d1")
                        nc.vector.tensor_scalar_add(d1[:], ps[:, HD:MW], EPS)
                        d2 = sm.tile([P, H], f32, tag="d2")
                        nc.vector.reciprocal(d2[:], d1[:])
                        nc.vector.tensor_scalar_mul(
                            inv_sb[:, i * H:(i + 1) * H], d2[:], 1.0 - ALPHA)
                    # epilogue: zn = ps * inv + alpha * ent
                    zn = wk.tile([P, HD], bf16, tag="zn")
                    inv_b = (inv_sb[:, i * H:(i + 1) * H]
                             .rearrange("p (h o) -> p h o", o=1)
                             .to_broadcast([P, H, D]))
                    nc.vector.tensor_tensor(
                        out=zn[:].rearrange("p (h d) -> p h d", h=H),
                        in0=ps[:, 0:HD].rearrange("p (h d) -> p h d", h=H),
                        in1=inv_b, op=OP.mult)
                    ent_b = (aent[:, i * D:(i + 1) * D]
                             .rearrange("p (o d) -> p o d", o=1)
                             .to_broadcast([P, H, D]))
                    zn3 = zn[:].rearrange("p (h d) -> p h d", h=H)
                    nc.vector.tensor_tensor(out=zn3, in0=zn3, in1=ent_b,
                                            op=OP.add)
                    if not last:
                        nc.scalar.dma_start(z_shard[i * P:(i + 1) * P, :],
                                          zn[:])
                        if i + 1 in GBOUND:
                            g = GBOUND.index(i + 1) - 1
                            nc.gpsimd.collective_compute(
                                "AllGather", mybir.AluOpType.bypass,
                                replica_groups=rg,
                                ins=[z_shard[RB[g]:RB[g + 1], :].opt()],
                                outs=[z_full[(k - 1) % 2]
                                      [NCORES * RB[g]:NCORES * RB[g + 1],
                                       :].opt()])
                    else:
                        po = ppC.tile([P, D], f32, tag="shr")
                        for b in range(HD // P):
                            tpp = ppB.tile([P, P], bf16, tag="tp")
                            nc.tensor.transpose(out=tpp[:],
                                                in_=zn[:, b * P:(b + 1) * P],
                                                identity=identb[:])
                            tps = wk.tile([P, P], bf16, tag="tps")
                            nc.scalar.activation(tps[:], tpp[:], AF.Copy)
                            nc.tensor.matmul(po[:, :], lhsT=tps[:],
                                             rhs=wo_t[:, b * D:(b + 1) * D],
                                             start=(b == 0),
                                             stop=(b == HD // P - 1))
                        ob = wk.tile([P, D], f32, tag="ob")
                        nc.vector.tensor_copy(ob[:], po[:, :])
                        nc.scalar.dma_start(out_d[i * P:(i + 1) * P, :], ob[:])
    nc.compile()
    return nc


def make_in_maps(cfg, cores, relpad_b, W_h, W_t, W_r, att_h, att_t,
                 att_r, W_o):
    import ml_dtypes

    bf16 = ml_dtypes.bfloat16

    def rep(att, n):
        a = np.concatenate([np.asarray(x, np.float32).reshape(1, HD)
                            for x in att], axis=1)
        return np.tile(a, (P, 1)).astype(bf16)

    wht = np.concatenate([np.asarray(W_h, np.float32),
                          np.asarray(W_t, np.float32)], axis=1).astype(bf16)
    wo = np.asarray(W_o, np.float32)  # [256, 64]
    wo_b = np.concatenate([wo[:P, :], wo[P:, :]], axis=1).astype(bf16)
    common = dict(
        relpad=np.ascontiguousarray(relpad_b),
        wht=np.ascontiguousarray(wht),
        wr=np.ascontiguousarray(np.asarray(W_r, np.float32).astype(bf16)),
        wo=np.ascontiguousarray(wo_b),
        attht=np.ascontiguousarray(rep([att_h, att_t], 2)),
        attr=np.ascontiguousarray(rep([att_r], 1)),
    )
    in_maps = []
    for c in range(NCORES):
        m = dict(common)
        m["entloc"] = np.ascontiguousarray(cores[c]["entloc"])
        m["tti"] = np.ascontiguousarray(cores[c]["tti"])
        m["s1h"] = np.ascontiguousarray(cores[c]["s1h"])
        m["h1h"] = np.ascontiguousarray(cores[c]["h1h"])
        m["r1h"] = np.ascontiguousarray(cores[c]["r1h"])
        in_maps.append(m)
    return in_maps


_CACHE = {}


def kernel(entity_embed, relation_embed, W_h, W_t, W_r, att_h, att_t, att_r,
           W_o, edge_index, edge_type):
    from concourse.bass_utils import run_bass_kernel_spmd

    cfg = Cfg(640)
    cfg, cores, relpad_b, newpos = host_prep(
        cfg, entity_embed, relation_embed, edge_index, edge_type)
    in_maps = make_in_maps(cfg, cores, relpad_b, W_h, W_t, W_r,
                           att_h, att_t, att_r, W_o)
    key = cfg.l_cap
    if key not in _CACHE:
        _CACHE[key] = build_program(cfg)
    nc = _CACHE[key]
    res = run_bass_kernel_spmd(nc, in_maps, core_ids=list(range(NCORES)))
    full = np.concatenate(
        [res.results[c]["out"] for c in range(NCORES)], axis=0)
    return full[newpos].astype(np.float32)
